# revision 24
# baseline (speedup 1.0000x reference)
"""DeepSeekV3 MoE gate (256 experts, top-8, 8 groups/top-4, sigmoid scoring,
seq-aux-loss) as a Bass/Tile kernel on 8 Trainium2 NeuronCores.

Strategy (data parallel over tokens, per the sharding hint):
  - 8192 tokens are split 1024/core (cores 0-3 hold batch row 0, 4-7 row 1);
    the [256, 7168] gate weight + [256] bias are replicated.
  - Per core, logits = x @ W.T are computed as three bf16 matmul products
    (xh@wh + xl@wh + xh@wl with x = xh + xl, W = wh + wl hi/lo bf16 splits)
    accumulated in fp32 PSUM — full-rate PE with ~fp32 accuracy.  The host
    pre-splits and pre-transposes x/W into PE-ready [h, t]/[h, e] tile
    layouts so no on-device transposes are needed.
  - Routing per 128-token tile entirely on-chip: sigmoid (ScalarE LUT),
    +bias, grouped top-2-sum via the DVE Max8 unit, top-4 group threshold +
    mask, masked top-8 values/indices (Max8 + MaxIndex — identical ordering
    and tie-break semantics to jax.lax.top_k), normalize, scale.
  - Aux-loss needs only two per-expert sums over each batch row (selection
    counts and sigmoid-score sums); each core reduces its tokens via a
    1-row fp16 matmul into PSUM and the host combines the 8 [2,256]
    partials into the scalar (the "all-reduce only the aux scalar" step).
"""

import numpy as np
import ml_dtypes

BF16 = ml_dtypes.bfloat16

E = 256        # experts
H = 7168       # hidden dim
G = 8          # groups
EG = E // G    # experts per group
TOPK = 8
TOPKG = 4
HC = H // 128  # contraction chunks
ROUTE_SCALE = 2.5
AUX_ALPHA = 0.001
NCORES = 8
NT = 8         # 128-token tiles per core

_cache = {}


def _build_module(with_bias=False):
    from concourse import bacc, tile, mybir

    f32 = mybir.dt.float32
    bf16 = mybir.dt.bfloat16
    f16 = mybir.dt.float16
    u32 = mybir.dt.uint32
    AF = mybir.ActivationFunctionType
    OP = mybir.AluOpType
    AX = mybir.AxisListType

    nc = bacc.Bacc("TRN2", target_bir_lowering=False, debug=False)

    xh_d = nc.dram_tensor("xh", [NT, 128, HC, 128], bf16, kind="ExternalInput")
    xl_d = nc.dram_tensor("xl", [NT, 128, HC, 128], bf16, kind="ExternalInput")
    wc_d = nc.dram_tensor("wc", [128, HC, 512], bf16, kind="ExternalInput")
    br_d = nc.dram_tensor("biasr", [128, E], f32, kind="ExternalInput")
    out_d = nc.dram_tensor("out", [NT, 128, 2 * TOPK], f32, kind="ExternalOutput")
    aux_d = nc.dram_tensor("aux", [1, E], f32, kind="ExternalOutput")

    with tile.TileContext(nc) as tc:
        with (
            tc.tile_pool(name="wpool", bufs=1) as wpool,
            tc.tile_pool(name="xpool", bufs=4) as xpool,
            tc.tile_pool(name="spool", bufs=2) as spool,
            tc.tile_pool(name="shpool", bufs=5) as shpool,
            tc.tile_pool(name="opool", bufs=2) as opool,
            tc.tile_pool(name="cpool", bufs=1) as cpool,
            tc.tile_pool(name="psum", bufs=4, space="PSUM") as psum_pool,
            tc.tile_pool(name="psum_aux", bufs=1, space="PSUM") as paux_pool,
            tc.tile_pool(name="psum_wu", bufs=1, space="PSUM") as pwu_pool,
        ):
            # PE clock warmup: the HAM gate holds the PE at 1.2 GHz until it
            # sees ~3.4us of sustained activity.  The first real matmul can't
            # start before its data lands (~6us), so spend that dead window on
            # dummy matmuls over zeroed scratch — the real matmuls then issue
            # at the full 2.4 GHz from the start.
            wu_m = cpool.tile([128, 512], bf16, tag="wu_m")
            nc.vector.memset(wu_m[:], 0.0)
            wu_ps = pwu_pool.tile([128, 512], f32, tag="wu_ps")
            for _ in range(10):
                nc.tensor.matmul(
                    wu_ps[:], lhsT=wu_m[:, 0:128], rhs=wu_m[:],
                    start=True, stop=True,
                )
            # Preamble: stream w chunk-slices interleaved with BOTH tile-0 and
            # tile-1 x slices, and emit their matmuls in the same slice order.
            # Early on the kernel is DMA-bound; unlocking two tiles per w-slice
            # raises PE-work-per-DMA-byte from 0.58 to 0.87 during the w phase.
            # uniform 7-chunk slice groups (a finer first group tested worse:
            # extra HWDGE descriptor serialization outweighs the earlier start)
            BOUNDS = [7 * k for k in range(9)]
            XSPLIT = 8
            xstep = HC // XSPLIT
            w_sb = wpool.tile([128, HC, 512], bf16)
            PRE = 3
            pre_x = []
            for m in range(PRE):
                xh_t = xpool.tile([128, HC, 128], bf16, tag="xh")
                xl_t = xpool.tile([128, HC, 128], bf16, tag="xl")
                pre_x.append((xh_t, xl_t))
            for s in range(len(BOUNDS) - 1):
                lo, hi = BOUNDS[s], BOUNDS[s + 1]
                nc.sync.dma_start(w_sb[:, lo:hi, :], wc_d[:, lo:hi, :])
                for m in range(PRE):
                    nc.sync.dma_start(pre_x[m][0][:, lo:hi, :], xh_d[m, :, lo:hi, :])
                    nc.sync.dma_start(pre_x[m][1][:, lo:hi, :], xl_d[m, :, lo:hi, :])
            if with_bias:
                bias_r = cpool.tile([128, E], f32)
                nc.sync.dma_start(bias_r[:], br_d[:])
            ones_h = cpool.tile([128, 1], f16)
            nc.vector.memset(ones_h[:], 1.0)

            sall_ps = paux_pool.tile([1, E], f32, tag="sall")
            # aux matmuls are deferred until after the NEXT tile's matmul block
            # so the PE never waits on the DVE-produced scores_h at a tile
            # boundary (the input is ~17us old by the time PE reaches it)
            pending_aux = []

            def flush_aux():
                while pending_aux:
                    an, sh = pending_aux.pop(0)
                    nc.tensor.matmul(
                        sall_ps[:], lhsT=ones_h[:], rhs=sh[:],
                        start=(an == 0), stop=(an == NT - 1),
                    )

            def emit_mms_512(ps, xh_t, xl_t, c, first, last):
                # [wh|wl] N=512 shares one LDWEIGHTS across two products:
                # P1[:, 0:256] += xh@wh (+ xl@wh), P2[:, 256:512] += xh@wl
                nc.tensor.matmul(
                    ps[:, :], lhsT=xh_t[:, c, :], rhs=w_sb[:, c, :],
                    start=first, stop=False,
                )
                nc.tensor.matmul(
                    ps[:, 0:E], lhsT=xl_t[:, c, :], rhs=w_sb[:, c, 0:E],
                    start=False, stop=last,
                )

            def route_tile(n, ps, sig_src):
                # sig_src: AP holding complete logits [128, E]
                scores = spool.tile([128, E], f32, tag="scores")
                nc.scalar.activation(scores[:], sig_src, AF.Sigmoid)

                # aux partial (score sums only -- selection counts are an exact
                # host-side bincount of the indices output): fp16 matmul,
                # emitted right after sigmoid so the PE-side accumulation never
                # waits on the top-k chain.
                scores_h = shpool.tile([128, E], f16, tag="scores_h")
                nc.vector.tensor_copy(scores_h[:], scores[:])
                pending_aux.append((n, scores_h))

                if with_bias:
                    scores_b = spool.tile([128, E], f32, tag="scores_b")
                    nc.vector.tensor_add(scores_b[:], scores[:], bias_r[:])
                else:
                    # bias is all-zero: biased scores == sigmoid scores
                    scores_b = scores

                # per-group top-2 sum -> group scores
                top8g = spool.tile([128, G, 8], f32, tag="top8g")
                for g in range(G):
                    nc.vector.max(top8g[:, g, :], scores_b[:, g * EG : (g + 1) * EG])
                gsc = spool.tile([128, G, 1], f32, tag="gsc")
                nc.vector.reduce_sum(gsc[:], top8g[:, :, 0:2], axis=AX.X)

                # top-4 groups -> masked scores, fused in one DVE op:
                # masked = (group_score >= 4th-largest) * scores
                g8 = spool.tile([128, 8], f32, tag="g8")
                nc.vector.max(g8[:], gsc[:, :, 0])
                masked = spool.tile([128, G, EG], f32, tag="masked")
                nc.vector.scalar_tensor_tensor(
                    masked[:],
                    gsc[:].broadcast_to([128, G, EG]),
                    g8[:, 3:4],
                    scores_b[:].rearrange("p (g e) -> p g e", g=G),
                    op0=OP.is_ge, op1=OP.mult,
                )
                masked2 = masked[:].rearrange("p g e -> p (g e)")

                # top-8 values (desc) + indices (lowest-index tie-break);
                # weights and indices share one output tile per 128-token
                # block ([:, 0:8] weights f32, [:, 8:16] index bits)
                v8 = spool.tile([128, 8], f32, tag="v8")
                nc.vector.max(v8[:], masked2)
                joint = opool.tile([128, 2 * TOPK], f32, tag="joint")
                nc.vector.max_index(joint[:, 8:16].bitcast(u32), v8[:], masked2)

                # weights = v8 / sum(v8) * ROUTE_SCALE
                s8 = spool.tile([128, 1], f32, tag="s8")
                nc.vector.reduce_sum(s8[:], v8[:], axis=AX.X)
                r8 = spool.tile([128, 1], f32, tag="r8")
                nc.vector.reciprocal(r8[:], s8[:])
                nc.vector.tensor_scalar(
                    joint[:, 0:8], v8[:], r8[:, 0:1], ROUTE_SCALE,
                    op0=OP.mult, op1=OP.mult,
                )
                nc.sync.dma_start(out_d[n], joint[:])

            def merge_and_route(n, ps):
                # P1 + P2 merge (ScalarE copy + DVE add), then routing
                p2c = spool.tile([128, E], f32, tag="p2c")
                nc.scalar.copy(p2c[:], ps[:, E : 2 * E])
                logits = spool.tile([128, E], f32, tag="logits_sb")
                nc.vector.tensor_add(logits[:], ps[:, 0:E], p2c[:])
                route_tile(n, ps=ps, sig_src=logits[:])

            # preamble tiles: matmuls interleaved in DMA slice-arrival order
            ps0 = psum_pool.tile([128, 512], f32, tag="logits")
            ps1 = psum_pool.tile([128, 512], f32, tag="logits")
            ps2 = psum_pool.tile([128, 512], f32, tag="logits")
            pre_ps = [ps0, ps1, ps2][:PRE]
            for s in range(len(BOUNDS) - 1):
                lo, hi = BOUNDS[s], BOUNDS[s + 1]
                for m in range(PRE):
                    xh_t, xl_t = pre_x[m]
                    for c in range(lo, hi):
                        emit_mms_512(
                            pre_ps[m], xh_t, xl_t, c,
                            first=(c == 0), last=(c == HC - 1),
                        )
            for m in range(PRE):
                merge_and_route(m, pre_ps[m])

            # remaining tiles: monolithic, x prefetched in 7-chunk slices
            for n in range(PRE, NT):
                xh_t = xpool.tile([128, HC, 128], bf16, tag="xh")
                xl_t = xpool.tile([128, HC, 128], bf16, tag="xl")
                for s in range(XSPLIT):
                    lo = s * xstep
                    hi = HC if s == XSPLIT - 1 else (s + 1) * xstep
                    nc.sync.dma_start(xh_t[:, lo:hi, :], xh_d[n, :, lo:hi, :])
                    nc.sync.dma_start(xl_t[:, lo:hi, :], xl_d[n, :, lo:hi, :])

                ps = psum_pool.tile([128, 512], f32, tag="logits")
                if n < NT - 1:
                    for c in range(HC):
                        emit_mms_512(ps, xh_t, xl_t, c, first=(c == 0), last=(c == HC - 1))
                    flush_aux()
                    merge_and_route(n, ps)
                else:
                    # LAST tile: no successor to hide the merge under, so use
                    # three N=256 matmuls all accumulating into P1 and skip the
                    # merge entirely
                    for c in range(HC):
                        nc.tensor.matmul(
                            ps[:, 0:E], lhsT=xh_t[:, c, :], rhs=w_sb[:, c, 0:E],
                            start=(c == 0), stop=False,
                        )
                        nc.tensor.matmul(
                            ps[:, 0:E], lhsT=xh_t[:, c, :], rhs=w_sb[:, c, E : 2 * E],
                            start=False, stop=False,
                        )
                        nc.tensor.matmul(
                            ps[:, 0:E], lhsT=xl_t[:, c, :], rhs=w_sb[:, c, 0:E],
                            start=False, stop=(c == HC - 1),
                        )
                    flush_aux()
                    route_tile(n, ps=ps, sig_src=ps[:, 0:E])

            flush_aux()
            aux_sb = opool.tile([1, E], f32, tag="aux_sb")
            nc.vector.tensor_copy(aux_sb[:], sall_ps[:])
            nc.sync.dma_start(aux_d[:], aux_sb[:])

    nc.compile()
    return nc


def _get_module(with_bias=False):
    key = f"nc{int(with_bias)}"
    if key not in _cache:
        _cache[key] = _build_module(with_bias=with_bias)
    return _cache[key]


def _host_prep_x(x2):
    """x2 [T, H] f32 -> per-core (xh, xl) [NT, 128, HC, 128] bf16 tile layouts."""
    tpc = x2.shape[0] // NCORES
    xh = x2.astype(BF16)
    xl = (x2 - xh.astype(np.float32)).astype(BF16)
    out = []
    for c in range(NCORES):
        sl = slice(c * tpc, (c + 1) * tpc)
        pair = []
        for a in (xh[sl], xl[sl]):
            b = a.reshape(NT, 128, HC, 128).transpose(0, 3, 2, 1)
            pair.append(np.ascontiguousarray(b))
        out.append(pair)
    return out


def _host_prep_w(w):
    """w [E, H] f32 -> [128, HC, 512] bf16 ([..., :256]=wh.T, [..., 256:]=wl.T)."""
    wh = w.astype(BF16)
    wl = (w - wh.astype(np.float32)).astype(BF16)
    wc = np.empty((128, HC, 2 * E), dtype=BF16)
    wc[:, :, :E] = wh.reshape(E, HC, 128).transpose(2, 1, 0)
    wc[:, :, E:] = wl.reshape(E, HC, 128).transpose(2, 1, 0)
    return np.ascontiguousarray(wc)


def kernel(x, gate_weight, bias):
    from concourse.bass_utils import run_bass_kernel_spmd

    x = np.asarray(x)
    gate_weight = np.asarray(gate_weight)
    bias = np.asarray(bias)
    with_bias = bool(np.any(bias != 0))
    nc = _get_module(with_bias=with_bias)
    B, S, _ = x.shape
    x2 = x.reshape(B * S, H)

    per_core = _host_prep_x(x2)
    wc = _host_prep_w(gate_weight)
    br = np.ascontiguousarray(
        np.broadcast_to(bias.astype(np.float32)[None, :], (128, E))
    )
    in_maps = [{"xh": xh, "xl": xl, "wc": wc, "biasr": br} for xh, xl in per_core]

    res = run_bass_kernel_spmd(nc, in_maps, core_ids=list(range(NCORES)))

    out = np.concatenate(
        [r["out"].reshape(-1, 2 * TOPK) for r in res.results], axis=0
    )
    weights = np.ascontiguousarray(out[:, 0:TOPK]).astype(np.float32)
    indices = np.ascontiguousarray(out[:, TOPK:]).view(np.int32).astype(np.int32)

    # aux loss: per-expert score sums from the device, selection counts as an
    # exact bincount of the indices (the "all-reduce the aux scalar" step)
    cpb = NCORES // B
    tpb = indices.shape[0] // B
    terms = []
    for b in range(B):
        ssc = np.zeros(E, np.float64)
        for c in range(cpb * b, cpb * (b + 1)):
            ssc += res.results[c]["aux"].reshape(-1)
        counts = np.bincount(
            indices[b * tpb : (b + 1) * tpb].ravel(), minlength=E
        ).astype(np.float64)
        terms.append(float((counts * ssc).sum() / ((S * TOPK / E) * S)))
    aux_loss = np.float32(np.mean(terms) * AUX_ALPHA)

    return weights, indices, aux_loss
